# revision 19
# baseline (speedup 1.0000x reference)
"""Trainium2 Bass kernel for nn_Block_2302102471059 (ragged_sequence).

Pipeline per NeuronCore (8-way shard by GRU group ownership):
  - Each core owns 16 of the 128 sequence groups -> 8192 "slots"
    (slot s = l*16 + g_local, node id = seq_ids[g, l]).
  - Graph aggregation (segment mean over in-edges) is computed per owned
    slot directly from the replicated in_feats table: edges are bucketed
    by destination on the host, gathered 128-at-a-time with indirect DMA,
    and reduced with one-hot matmuls on the PE.
  - conv + ff1 run in a transposed layout (features on partitions,
    slots on the free dim), which is exactly what the GRU input matmul
    wants -- no layout changes anywhere in the main pipeline.
  - The GRU recurrence runs serially over L=512 steps with batch 16 in a
    transposed layout: h^T [128 part = 4 d-chunks x ..., 16 groups free].
    W_hh/W_ih live in SBUF as fp16 128x128 lhsT tiles (FWL-eligible).
  - ff2 + transpose back to row layout + contiguous store of a compact
    [8192, 512] fp32 per-core output; the host scatters rows into the
    full [65536, 512] output by seq_ids.

Everything on-device is fp16 storage with fp32 PSUM accumulation.
"""

import os
import sys

import numpy as np

sys.path.insert(0, "/opt/trn_rl_repo")

from contextlib import ExitStack

import concourse.bacc as bacc
import concourse.bass as bass
import concourse.tile as tile
from concourse import mybir
from concourse.bass import IndirectOffsetOnAxis
from concourse.bass_utils import run_bass_kernel_spmd
from concourse.masks import make_identity

N, D, E, G, L = 65536, 512, 1048576, 128, 512
NCORES = 8
GP = G // NCORES          # 16 groups per core
S = GP * L                # 8192 slots per core
NT = S // 128             # 64 dst-tiles of 128 slots
LCH = 32                  # l-steps per stream chunk
NCH = L // LCH            # 16 chunks
SCH = LCH * GP            # 512 slots per chunk
F16 = mybir.dt.float16
F32 = mybir.dt.float32
F32R = mybir.dt.float32r
I32 = mybir.dt.int32

LAST_RESULT = None
LAST_NC = None


def _build(nsub, debug=False):
    """Build the Bass program. nsub = padded 128-edge sub-blocks per dst-tile
    (uniform across cores/tiles; must be a multiple of 6)."""
    nc = bacc.Bacc("TRN2", target_bir_lowering=False, debug=False)
    calls_per_tile = nsub // 6

    # ---- DRAM I/O ----
    feats = nc.dram_tensor("in_feats", [N, D], F32, kind="ExternalInput")
    w_conv = nc.dram_tensor("W_conv", [D, D], F32, kind="ExternalInput")
    b_conv = nc.dram_tensor("b_conv", [D], F32, kind="ExternalInput")
    w_ff1 = nc.dram_tensor("W_ff1", [D, D], F32, kind="ExternalInput")
    b_ff1 = nc.dram_tensor("b_ff1", [D], F32, kind="ExternalInput")
    w_ih = nc.dram_tensor("W_ih", [3 * D, D], F32, kind="ExternalInput")
    w_hh = nc.dram_tensor("W_hh", [3 * D, D], F32, kind="ExternalInput")
    b_ih = nc.dram_tensor("b_ih", [3 * D], F32, kind="ExternalInput")
    b_hh = nc.dram_tensor("b_hh", [3 * D], F32, kind="ExternalInput")
    w_ff2 = nc.dram_tensor("W_ff2", [D, D], F32, kind="ExternalInput")
    b_ff2 = nc.dram_tensor("b_ff2", [D], F32, kind="ExternalInput")
    # Per-core index/meta arrays (host-prepared).
    # idx2d[p, t*nsub + q] = src node of padded edge slot (t, q, p)
    idx2d = nc.dram_tensor("idx2d", [128, NT * nsub], I32, kind="ExternalInput")
    ldst2d = nc.dram_tensor("ldst2d", [128, NT * nsub], I32, kind="ExternalInput")
    deg2d = nc.dram_tensor("deg2d", [128, NT], F32, kind="ExternalInput")
    out = nc.dram_tensor("out", [S, D], F32, kind="ExternalOutput")
    if debug:
        dbg_aggh = nc.dram_tensor("dbg_aggh", [128, 2048], F16, kind="ExternalOutput")
        dbg_mt = nc.dram_tensor("dbg_mt", [128, 2048], F16, kind="ExternalOutput")
        dbg_git = nc.dram_tensor("dbg_git", [128, LCH * 192], F16, kind="ExternalOutput")
        dbg_ring = nc.dram_tensor("dbg_ring", [128, 4096], F16, kind="ExternalOutput")

    with tile.TileContext(nc) as tc, ExitStack() as ctx:
        wpool = ctx.enter_context(tc.tile_pool(name="w", bufs=1))
        tmp = ctx.enter_context(tc.tile_pool(name="tmp", bufs=2))
        stage = ctx.enter_context(tc.tile_pool(name="stage", bufs=2))
        ohp = ctx.enter_context(tc.tile_pool(name="oh", bufs=2))
        aggp = ctx.enter_context(tc.tile_pool(name="agg", bufs=2))
        xtp = ctx.enter_context(tc.tile_pool(name="xt", bufs=2))
        ctp = ctx.enter_context(tc.tile_pool(name="ct", bufs=2))
        mtp = ctx.enter_context(tc.tile_pool(name="mt", bufs=2))
        gip = ctx.enter_context(tc.tile_pool(name="gi", bufs=2))
        grup = ctx.enter_context(tc.tile_pool(name="gru", bufs=2))
        outp = ctx.enter_context(tc.tile_pool(name="outw", bufs=2))
        ps_mm = ctx.enter_context(tc.tile_pool(name="psmm", bufs=3, space="PSUM"))
        ps_gi = ctx.enter_context(tc.tile_pool(name="psgi", bufs=2, space="PSUM"))
        ps_gru = ctx.enter_context(tc.tile_pool(name="psgru", bufs=1, space="PSUM"))
        ps_tr = ctx.enter_context(tc.tile_pool(name="pstr", bufs=2, space="PSUM"))

        # ---- constants / weights prep ----
        ident = wpool.tile([128, 128], F16, tag="ident")
        make_identity(nc, ident[:])
        iotaf = wpool.tile([128, 128], F32, tag="iotaf")
        iotai = tmp.tile([128, 128], I32, tag="ioi")
        nc.gpsimd.iota(iotai[:], pattern=[[1, 128]], base=0, channel_multiplier=0)
        nc.vector.tensor_copy(iotaf[:], iotai[:])
        zero64 = wpool.tile([128, 64], F16, tag="zero64")
        nc.vector.memset(zero64[:], 0.0)

        def load_cast(dram, rows, cols, tag):
            t = wpool.tile([128, cols * (rows // 128)], F16, tag=tag)
            for c in range(rows // 128):
                f = tmp.tile([128, cols], F32, tag="ldf")
                nc.gpsimd.dma_start(out=f[:], in_=dram[c * 128:(c + 1) * 128, :])
                nc.vector.tensor_copy(t[:, c * cols:(c + 1) * cols], f[:])
            return t

        # lhsT tiles: [k, m] with k = input feature. W_conv/W_ff1/W_ff2 are
        # already [in, out]; c-th row-block is the c-th k-chunk.
        wconv = load_cast(w_conv, D, D, "wconv")    # [:, c*512+m]
        wff1 = load_cast(w_ff1, D, D, "wff1")
        wff2 = load_cast(w_ff2, D, D, "wff2")

        def load_gate_T(dram, tag):
            # W [1536, 512] (gate-major rows) -> lhsT tiles [k=d, m=gate],
            # stored as [128, 4c * 1536] : col c*1536 + j*128 + m
            t = wpool.tile([128, 4 * 1536], F16, tag=tag)
            for j in range(12):
                f = tmp.tile([128, 512], F32, tag="ldf")
                nc.gpsimd.dma_start(out=f[:], in_=dram[j * 128:(j + 1) * 128, :])
                h = tmp.tile([128, 512], F16, tag="ldh")
                nc.vector.tensor_copy(h[:], f[:])
                for c in range(4):
                    pt = ps_tr.tile([128, 128], F16, space="PSUM", tag="ptr")
                    nc.tensor.transpose(pt[:], h[:, c * 128:(c + 1) * 128], ident[:])
                    nc.scalar.activation(
                        t[:, c * 1536 + j * 128: c * 1536 + (j + 1) * 128], pt[:],
                        mybir.ActivationFunctionType.Copy)
            return t

        wihT = load_gate_T(w_ih, "wihT")
        whhT = load_gate_T(w_hh, "whhT")

        def load_bias(dram, n, tag):
            # [n*128] -> [128, n]
            t = wpool.tile([128, n], F32, tag=tag)
            for m in range(n):
                nc.gpsimd.dma_start(out=t[:, m:m + 1],
                                  in_=dram[m * 128:(m + 1) * 128][:, None])
            return t

        bconv = load_bias(b_conv, 4, "bconv")
        bff1 = load_bias(b_ff1, 4, "bff1")
        bih = load_bias(b_ih, 12, "bih")
        bhh = load_bias(b_hh, 12, "bhh")
        bff2 = load_bias(b_ff2, 4, "bff2")

        # gate biases: the PSUM fuse adds gi (which already contains b_ih);
        # b_hh must be added too for r/z/n. Fold b_hh into the gi eviction
        # bias: total bias = b_ih + b_hh (both added to every step's gates).
        # NOTE: n-gate: reference computes inn + r*(hn) with hn including
        # b_hh_n. So b_hh_n must stay with gh_n (PSUM side), NOT folded.
        # r/z: sigmoid(gi_r + gh_r + b_ih_r + b_hh_r) -> fold b_hh_rz into
        # gi eviction; add b_hh_n to psum n-region via the gates path.
        bsum = wpool.tile([128, 12], F32, tag="bsum")
        nc.vector.tensor_add(bsum[:, 0:8], bih[:, 0:8], bhh[:, 0:8])
        nc.vector.tensor_copy(bsum[:, 8:12], bih[:, 8:12])
        # b_hh_n broadcast into h-layout [128, 64]: [p, c*16+b] = b_hh[1024+c*128+p]
        bhn = wpool.tile([128, 64], F32, tag="bhn")
        for c in range(4):
            bc = bhh[:, 8 + c:9 + c]
            b3 = bass.AP(bc.tensor, bc.offset, [bc.ap[0], [0, 16]])
            nc.vector.tensor_copy(bhn[:, c * 16:(c + 1) * 16], b3)

        # per-slot inverse degree
        degs = wpool.tile([128, NT], F32, tag="degs")
        nc.sync.dma_start(out=degs[:], in_=deg2d[:, :])
        degm = wpool.tile([128, NT], F32, tag="degm")
        nc.vector.tensor_scalar_max(degm[:], degs[:], 1.0)
        invdeg = wpool.tile([128, NT], F32, tag="invdeg")
        nc.vector.reciprocal(invdeg[:], degm[:])

        # edge meta
        idx_sb = wpool.tile([128, NT * nsub], I32, tag="idxsb")
        nc.sync.dma_start(out=idx_sb[:], in_=idx2d[:, :])
        ldst_i = tmp.tile([128, NT * nsub], I32, tag="ldsti")
        nc.sync.dma_start(out=ldst_i[:], in_=ldst2d[:, :])
        ldst_f = wpool.tile([128, NT * nsub], F32, tag="ldstf")
        nc.vector.tensor_copy(ldst_f[:], ldst_i[:])

        # GRU hidden ring buffer: 64 l-slots x [4 d-chunks x 16 groups]
        ring = wpool.tile([128, 64 * 64], F16, tag="ring")

        def gru_step(t_step, gi_t, gi_base):
            """One GRU step. gi_t: gi chunk tile; gi_base: col offset of this
            step's [128,192] block. Writes h_t into ring slot t_step%64."""
            if t_step == 0:
                h_prev = zero64[:]
            else:
                o = ((t_step - 1) % 64) * 64
                h_prev = ring[:, o:o + 64]
            ps = ps_gru.tile([128, 192], F32, space="PSUM", tag="psg")
            # u_rz = gi_rz (+b_hh_rz folded already) ... identity matmul first
            nc.tensor.matmul(ps[:, 0:128], ident[:], gi_t[:, gi_base:gi_base + 128],
                             start=True, stop=False)
            for j in range(8):
                for c in range(4):
                    nc.tensor.matmul(
                        ps[:, j * 16:(j + 1) * 16],
                        whhT[:, c * 1536 + j * 128: c * 1536 + (j + 1) * 128],
                        h_prev[:, c * 16:(c + 1) * 16],
                        start=False, stop=(c == 3))
            for j in range(8, 12):
                for c in range(4):
                    nc.tensor.matmul(
                        ps[:, j * 16:(j + 1) * 16],
                        whhT[:, c * 1536 + j * 128: c * 1536 + (j + 1) * 128],
                        h_prev[:, c * 16:(c + 1) * 16],
                        start=(c == 0), stop=(c == 3))
            # gates
            sig = grup.tile([128, 128], F16, tag="sig")
            nc.scalar.activation(sig[:], ps[:, 0:128],
                                 mybir.ActivationFunctionType.Sigmoid)
            # hn = gh_n + b_hh_n ; rhn = r * hn
            hn = grup.tile([128, 64], F16, tag="hn")
            nc.vector.tensor_add(hn[:], ps[:, 128:192], bhn[:])
            rhn = grup.tile([128, 64], F16, tag="rhn")
            nc.vector.tensor_mul(rhn[:], sig[:, 0:64], hn[:])
            tg = grup.tile([128, 64], F16, tag="tg")
            nc.vector.tensor_add(tg[:], rhn[:], gi_t[:, gi_base + 128:gi_base + 192])
            n_t = grup.tile([128, 64], F16, tag="nt")
            nc.scalar.activation(n_t[:], tg[:], mybir.ActivationFunctionType.Tanh)
            zh = grup.tile([128, 64], F16, tag="zh")
            nc.vector.tensor_mul(zh[:], sig[:, 64:128], h_prev)
            omz = grup.tile([128, 64], F16, tag="omz")
            nc.scalar.activation(omz[:], sig[:, 64:128],
                                 mybir.ActivationFunctionType.Copy,
                                 bias=1.0, scale=-1.0)
            mm_ = grup.tile([128, 64], F16, tag="mm")
            nc.vector.tensor_mul(mm_[:], omz[:], n_t[:])
            hslot = ring[:, (t_step % 64) * 64:(t_step % 64) * 64 + 64]
            nc.vector.tensor_add(hslot, mm_[:], zh[:])

        def ff2_block(k):
            """slots [512k, 512k+512) = l in [32k, 32k+32); reads ring."""
            l0 = (LCH * k) % 64
            rr = ring[:].rearrange("p (l q) -> p l q", q=64)
            ot = outp.tile([128, 4 * 512], F16, tag="ot")
            for m in range(4):
                ps = ps_mm.tile([128, 512], F32, space="PSUM", tag="ps512")
                for c in range(4):
                    nc.tensor.matmul(
                        ps[:], wff2[:, c * 512 + m * 128: c * 512 + (m + 1) * 128],
                        rr[:, l0:l0 + LCH, c * 16:(c + 1) * 16],
                        start=(c == 0), stop=(c == 3))
                nc.scalar.activation(ot[:, m * 512:(m + 1) * 512], ps[:],
                                     mybir.ActivationFunctionType.Identity,
                                     bias=bff2[:, m:m + 1])
            for q in range(4):
                orow = outp.tile([128, 512], F32, tag="orow")
                for m in range(4):
                    pt = ps_tr.tile([128, 128], F16, space="PSUM", tag="ptr")
                    nc.tensor.transpose(pt[:], ot[:, m * 512 + q * 128: m * 512 + (q + 1) * 128],
                                        ident[:])
                    nc.vector.tensor_copy(orow[:, m * 128:(m + 1) * 128], pt[:])
                nc.gpsimd.dma_start(out=out[k * 512 + q * 128: k * 512 + (q + 1) * 128, :],
                                  in_=orow[:])

        # ================= streaming main pipeline =================
        for k in range(NCH):
            # ---- aggregation for the chunk's 4 dst-tiles ----
            xt = xtp.tile([128, 4 * SCH], F16, tag="xt")  # [c*512 + s_local]
            for tt in range(4):
                tg = 4 * k + tt  # global dst-tile
                psa = ps_mm.tile([128, 512], F32, space="PSUM", tag="ps512")
                oh = ohp.tile([128, nsub * 128], F32R, tag="oh")
                for cc in range(calls_per_tile):
                    st = stage.tile([128, 6 * 512], F32R, tag="st")
                    for i6 in range(6):
                        qq = cc * 6 + i6
                        nc.gpsimd.indirect_dma_start(
                            out=st[:, i6 * 512:(i6 + 1) * 512], out_offset=None,
                            in_=feats[:, :],
                            in_offset=IndirectOffsetOnAxis(
                                ap=idx_sb[:, tg * nsub + qq: tg * nsub + qq + 1],
                                axis=0))
                    # one-hot for these 6 sub-blocks in one DVE op
                    src = ldst_f[:, tg * nsub + cc * 6: tg * nsub + cc * 6 + 6]
                    src3 = bass.AP(src.tensor, src.offset,
                                   [src.ap[0], src.ap[1], [0, 128]])
                    io3 = bass.AP(iotaf[:].tensor, iotaf[:].offset,
                                  [iotaf[:].ap[0], [0, 6], iotaf[:].ap[1]])
                    oh3 = oh[:, cc * 768:(cc + 1) * 768].rearrange(
                        "p (q m) -> p q m", m=128)
                    nc.vector.tensor_tensor(out=oh3, in0=src3, in1=io3,
                                            op=mybir.AluOpType.is_equal)
                    for i in range(6):
                        q = cc * 6 + i
                        nc.tensor.matmul(
                            psa[:], oh[:, q * 128:(q + 1) * 128],
                            st[:, i * 512:(i + 1) * 512],
                            start=(q == 0), stop=(q == nsub - 1))
                aggh = aggp.tile([128, 512], F16, tag="aggh")
                nc.vector.tensor_scalar(out=aggh[:], in0=psa[:],
                                        scalar1=invdeg[:, tg:tg + 1], scalar2=None,
                                        op0=mybir.AluOpType.mult)
                if debug and k == 0:
                    nc.gpsimd.dma_start(out=dbg_aggh[:, tt * 512:(tt + 1) * 512],
                                        in_=aggh[:])
                for c in range(4):
                    pt = ps_tr.tile([128, 128], F16, space="PSUM", tag="ptr")
                    nc.tensor.transpose(pt[:], aggh[:, c * 128:(c + 1) * 128], ident[:])
                    nc.scalar.activation(xt[:, c * 512 + tt * 128: c * 512 + (tt + 1) * 128],
                                         pt[:], mybir.ActivationFunctionType.Copy)
            # ---- conv ----
            ct = ctp.tile([128, 4 * SCH], F16, tag="ct")
            for m in range(4):
                ps = ps_mm.tile([128, 512], F32, space="PSUM", tag="ps512")
                for c in range(4):
                    nc.tensor.matmul(ps[:],
                                     wconv[:, c * 512 + m * 128: c * 512 + (m + 1) * 128],
                                     xt[:, c * 512:(c + 1) * 512],
                                     start=(c == 0), stop=(c == 3))
                nc.scalar.activation(ct[:, m * 512:(m + 1) * 512], ps[:],
                                     mybir.ActivationFunctionType.Identity,
                                     bias=bconv[:, m:m + 1])
            # ---- ff1 (relu) ----
            mt = mtp.tile([128, 4 * SCH], F16, tag="mt")
            for m in range(4):
                ps = ps_mm.tile([128, 512], F32, space="PSUM", tag="ps512")
                for c in range(4):
                    nc.tensor.matmul(ps[:],
                                     wff1[:, c * 512 + m * 128: c * 512 + (m + 1) * 128],
                                     ct[:, c * 512:(c + 1) * 512],
                                     start=(c == 0), stop=(c == 3))
                nc.scalar.activation(mt[:, m * 512:(m + 1) * 512], ps[:],
                                     mybir.ActivationFunctionType.Relu,
                                     bias=bff1[:, m:m + 1])
            if debug and k == 0:
                nc.gpsimd.dma_start(out=dbg_mt[:, :], in_=mt[:])
            # ---- gi for the chunk: [128, LCH*192], col l*192 + j*16 + b ----
            git = gip.tile([128, LCH * 192], F16, tag="git")
            gir = git[:].rearrange("p (l j b) -> p l j b", j=12, b=16)
            for rep in range(2):
                for j in range(12):
                    ps = ps_gi.tile([128, 256], F32, space="PSUM", tag="psgi")
                    for c in range(4):
                        nc.tensor.matmul(
                            ps[:], wihT[:, c * 1536 + j * 128: c * 1536 + (j + 1) * 128],
                            mt[:, c * 512 + rep * 256: c * 512 + rep * 256 + 256],
                            start=(c == 0), stop=(c == 3))
                    nc.scalar.activation(
                        gir[:, rep * 16:(rep + 1) * 16, j, :],
                        ps[:].rearrange("p (l b) -> p l b", b=16),
                        mybir.ActivationFunctionType.Identity,
                        bias=bsum[:, j:j + 1])
            if debug and k == 0:
                nc.gpsimd.dma_start(out=dbg_git[:, :], in_=git[:])
            # ---- 32 GRU steps ----
            for li in range(LCH):
                gru_step(k * LCH + li, git, li * 192)
            if debug and k == 1:
                nc.gpsimd.dma_start(out=dbg_ring[:, :], in_=ring[:])
            # ---- ff2 for the previous chunk's slots (ring safety: current
            # chunk k's ring writes are slots [32k..32k+32)%64; block k reads
            # the same — emit after steps so data is ready ----
            ff2_block(k)

    nc.compile()
    return nc


def _host_prep(inputs):
    """Bucket edges by destination slot per core; build per-core arrays."""
    seq_ids = np.asarray(inputs["seq_ids"]).astype(np.int64)
    edge_src = np.asarray(inputs["edge_src"]).astype(np.int64)
    edge_dst = np.asarray(inputs["edge_dst"]).astype(np.int64)

    counts = np.bincount(edge_dst, minlength=N)
    order = np.argsort(edge_dst, kind="stable")
    src_sorted = edge_src[order].astype(np.int32)
    rowptr = np.zeros(N + 1, dtype=np.int64)
    np.cumsum(counts, out=rowptr[1:])

    # slot -> node per core: slot s = l*GP + g_local
    # node = seq_ids[16c + g_local, l]
    slot_nodes = []
    for c in range(NCORES):
        sn = seq_ids[c * GP:(c + 1) * GP, :].T.reshape(-1)  # [S]
        slot_nodes.append(sn)

    # per dst-tile max edges -> nsub (uniform, multiple of 6)
    max_tile = 0
    tile_edges = []
    for c in range(NCORES):
        sn = slot_nodes[c]
        cnt = counts[sn]  # [S]
        te = cnt.reshape(NT, 128).sum(axis=1)
        tile_edges.append((cnt, te))
        max_tile = max(max_tile, int(te.max()))
    nsub = -(-max_tile // 768) * 6  # ceil to multiple of 6 sub-blocks
    nsub = max(nsub, 6)

    per_core = []
    for c in range(NCORES):
        sn = slot_nodes[c]
        cnt, te = tile_edges[c]
        idx2d = np.zeros((128, NT * nsub), dtype=np.int32)
        ldst2d = np.full((128, NT * nsub), 200, dtype=np.int32)
        for t in range(NT):
            nodes = sn[t * 128:(t + 1) * 128]
            k = int(te[t])
            srcs = np.empty(k, dtype=np.int32)
            ld = np.empty(k, dtype=np.int32)
            pos = 0
            for p in range(128):
                v = nodes[p]
                d = int(cnt[t * 128 + p])
                srcs[pos:pos + d] = src_sorted[rowptr[v]:rowptr[v] + d]
                ld[pos:pos + d] = p
                pos += d
            # pack into [p, q] with flat index q*128 + p
            npad = nsub * 128
            sp = np.zeros(npad, dtype=np.int32)
            lp = np.full(npad, 200, dtype=np.int32)
            sp[:k] = srcs
            lp[:k] = ld
            idx2d[:, t * nsub:(t + 1) * nsub] = sp.reshape(nsub, 128).T
            ldst2d[:, t * nsub:(t + 1) * nsub] = lp.reshape(nsub, 128).T
        deg2d = cnt.reshape(NT, 128).T.astype(np.float32)
        per_core.append({"idx2d": idx2d, "ldst2d": ldst2d, "deg2d": deg2d,
                         "slot_nodes": sn})
    return per_core, nsub


def kernel(**inputs):
    global LAST_RESULT, LAST_NC
    per_core, nsub = _host_prep(inputs)
    nc = _build(nsub)
    LAST_NC = nc

    shared = {}
    for name in ["in_feats", "W_conv", "b_conv", "W_ff1", "b_ff1", "W_ih",
                 "W_hh", "b_ih", "b_hh", "W_ff2", "b_ff2"]:
        shared[name] = np.ascontiguousarray(
            np.asarray(inputs[name]).astype(np.float32))

    in_maps = []
    for c in range(NCORES):
        m = dict(shared)
        m["idx2d"] = per_core[c]["idx2d"]
        m["ldst2d"] = per_core[c]["ldst2d"]
        m["deg2d"] = per_core[c]["deg2d"]
        in_maps.append(m)

    res = run_bass_kernel_spmd(nc, in_maps, list(range(NCORES)),
                               trace=bool(int(os.environ.get("KTRACE", "0"))))
    LAST_RESULT = res

    out_full = np.empty((N, D), dtype=np.float32)
    for c in range(NCORES):
        out_full[per_core[c]["slot_nodes"]] = res.results[c]["out"]
    return out_full
